# revision 36
# baseline (speedup 1.0000x reference)
"""Trainium2 Bass kernel for ClassicAttention (B=2, S=2048, D=1024, H=16).

Sharding: batch x head tensor parallel. Cores 0-3 own batch 0, cores 4-7
batch 1; within a 4-core group each core owns 4 heads (256 of 1024 dims).

Host-side (free): x pre-transposed to x^T per batch and pre-cast to bf16;
weights pre-sliced/cast; softmax scale folded into wq/bq; k-bias dropped
(exact softmax invariance); v-bias folded into the c_proj bias.

On-chip per core:
  - QKV: d-major Q^T,K^T for its 4 heads over its batch; V row-major.
  - Attention: transposed-scores S^T[k,q]; exp on ACT (additive -30 mask
    pre-exp on diagonal tiles); AV col-packed 2 heads/matmul (M=64);
    softmax denominators via col-tiled M=1 ones-matmuls (4 heads
    concurrent); normalize with reciprocal + gpsimd partition_broadcast.
  - Per q-super (512 rows): ctx AllGather within the 4-core batch group,
    c_proj deferred one super for overlap; output transposed [256, 2048].
All matmuls bf16 with fp32 PSUM accumulation.
"""

import numpy as np
import ml_dtypes

import concourse.bass as bass
import concourse.tile as tile
import concourse.mybir as mybir
from concourse import bacc, library_config
from concourse.bass_utils import run_bass_kernel_spmd

F32 = mybir.dt.float32
BF16 = mybir.dt.bfloat16

NCORES = 8
B, S, D = 2, 2048, 1024
H, HD = 16, 64
HPC = 4                    # heads per core
G = 4                      # q-supers of 512 per batch
KCH = D // 128             # 8 contraction chunks
NST = S // 128             # 16 s-tiles
EXP = mybir.ActivationFunctionType.Exp
DEBUG_TAPS = False


def build_ir(nc):
    # ---------------- DRAM I/O ----------------
    xT = nc.dram_tensor("xT", [D, S], BF16, kind="ExternalInput").ap()
    wqk = nc.dram_tensor("wqk", [D, 512], BF16, kind="ExternalInput").ap()
    wv = nc.dram_tensor("wv", [D, 256], BF16, kind="ExternalInput").ap()
    wp = nc.dram_tensor("wp", [D, 256], BF16, kind="ExternalInput").ap()
    bq = nc.dram_tensor("bq", [256], F32, kind="ExternalInput").ap()
    bp = nc.dram_tensor("bp", [256], F32, kind="ExternalInput").ap()
    outT = nc.dram_tensor("outT", [256, S], F32, kind="ExternalOutput").ap()
    taps = None
    if DEBUG_TAPS:
        taps = {
            "dbg_q": nc.dram_tensor("dbg_q", [128, 2, S], BF16,
                                    kind="ExternalOutput").ap(),
            "dbg_k": nc.dram_tensor("dbg_k", [128, 2, S], BF16,
                                    kind="ExternalOutput").ap(),
            "dbg_v": nc.dram_tensor("dbg_v", [128, NST, 256], BF16,
                                    kind="ExternalOutput").ap(),
            "dbg_cs": nc.dram_tensor("dbg_cs", [G, 128, 2, 512], BF16,
                                     kind="ExternalOutput").ap(),
            "dbg_sums": nc.dram_tensor("dbg_sums", [G, 128, 512], F32,
                                       kind="ExternalOutput").ap(),
            "dbg_ctxall": nc.dram_tensor("dbg_ctxall", [G, 1040, 512], BF16,
                                         kind="ExternalOutput").ap(),
        }

    # additive causal mask for diagonal tiles, two head-copies side by side:
    # mask[k, 128*a + j] = 0 if j >= k else -30
    tri = np.where(np.arange(128)[None, :] >= np.arange(128)[:, None],
                   0.0, -30.0).astype(np.float32)
    mask_np = np.concatenate([tri, tri], axis=1)  # [128, 256]
    mask_const = nc.inline_tensor(mask_np, "mask_const").ap()

    rg = [[0, 1, 2, 3], [4, 5, 6, 7]]

    with tile.TileContext(nc) as tc:
        _emit(nc, tc, xT, wqk, wv, wp, bq, bp, outT, mask_const, rg, taps)
    return nc


def _emit(nc, tc, xT, wqk, wv, wp, bq, bp, outT, mask_const, rg, taps=None):
    import contextlib
    es = contextlib.ExitStack()
    with es:
        singles = es.enter_context(tc.tile_pool(name="singles", bufs=1))
        dram = es.enter_context(tc.tile_pool(name="dram", bufs=1, space="DRAM"))

        # ------------- persistent SBUF -------------
        xt_sb = singles.tile([128, KCH, S], BF16)
        wqk_sb = singles.tile([128, KCH, 512], BF16)
        wv_sb = singles.tile([128, KCH, 256], BF16)
        wp_sb = singles.tile([128, KCH, 256], BF16)
        qT = singles.tile([128, 2, S], BF16)      # [d%128, head-group, q]
        kT = singles.tile([128, 2, S], BF16)
        v_sb = singles.tile([128, NST, 256], BF16)  # [s%128, s-tile, 4 heads x 64]
        bq_sb = singles.tile([128, 2], F32)
        bp_sb = singles.tile([128, 2], F32)
        mask_sb = singles.tile([128, 2, 128], F32)
        ones_sb = singles.tile([128, 1], BF16)
        ones_row = singles.tile([128, 64], F32)

        # DMA priority order: first QKV round needs wqk + xt chunk 0 + wv
        nc.vector.memset(ones_sb, 1.0)
        nc.vector.memset(ones_row, 1.0)
        xT_r = xT.rearrange("(c p) s -> p c s", p=128)
        wqk_r = wqk.rearrange("(c p) j -> p c j", p=128)
        nc.sync.dma_start(out=wqk_sb[:, 0:4, :], in_=wqk_r[:, 0:4, :])
        nc.sync.dma_start(out=xt_sb[:, 0:4, 0:512], in_=xT_r[:, 0:4, 0:512])
        nc.sync.dma_start(out=wqk_sb[:, 4:8, :], in_=wqk_r[:, 4:8, :])
        nc.sync.dma_start(out=xt_sb[:, 4:8, 0:512], in_=xT_r[:, 4:8, 0:512])
        nc.sync.dma_start(out=wv_sb, in_=wv.rearrange("(c p) j -> p c j", p=128))
        nc.sync.dma_start(out=bq_sb, in_=bq.rearrange("(t p) -> p t", p=128))
        for sb in range(1, G):
            nc.sync.dma_start(out=xt_sb[:, :, sb * 512:(sb + 1) * 512],
                              in_=xT_r[:, :, sb * 512:(sb + 1) * 512])
        nc.sync.dma_start(out=mask_sb, in_=mask_const.rearrange(
            "p (a j) -> p a j", a=2))
        nc.sync.dma_start(out=wp_sb, in_=wp.rearrange("(c p) j -> p c j", p=128))
        nc.sync.dma_start(out=bp_sb, in_=bp.rearrange("(t p) -> p t", p=128))

        # DRAM tiles for ctx exchange (normalized producer-side; 256 rows
        # per rank). Supers 1 and 2 get their own AllGather (overlapped
        # under later rounds); supers 3 and 0 share one merged tail gather.
        ctx_local = {g: dram.tile([256, 512], BF16, tag=f"ctxl{g}",
                                  name=f"ctxl{g}") for g in range(G)}
        ctx_all = {g: dram.tile([1024, 512], BF16, tag=f"ctxa{g}",
                                name=f"ctxa{g}") for g in range(G)}

        # ------------- pools -------------
        ps_m = es.enter_context(tc.tile_pool(name="ps_m", bufs=2, space="PSUM"))
        ps_av = es.enter_context(tc.tile_pool(name="ps_av", bufs=1, space="PSUM"))
        ps_sum = es.enter_context(tc.tile_pool(name="ps_sum", bufs=1, space="PSUM"))
        ps_cp = es.enter_context(tc.tile_pool(name="ps_cp", bufs=1, space="PSUM"))
        pt_pool = es.enter_context(tc.tile_pool(name="pt", bufs=6))
        post = es.enter_context(tc.tile_pool(name="post", bufs=2))
        ctxg_pool = es.enter_context(tc.tile_pool(name="ctxg", bufs=2))
        osb = es.enter_context(tc.tile_pool(name="osb", bufs=2))

        def qkv_chunks(g, pool=None):
            # fillers go through the single-bank ps_cp pool so their
            # matmuls never hold the score-psum slots; upfront chunks use
            # the 2-slot ps_m pool for pipelining
            def emit_qk_half(jt, half, dest, biased):
                if pool is ps_m:
                    ps = pool.tile([128, 2, 512], F32, tag="m",
                                   name=f"qk{jt}_{half}_{g}")[:, 0, :]
                else:
                    ps = ps_cp.tile([128, 512], F32, tag="cp",
                                    name=f"qk{jt}_{half}_{g}")
                col = jt * 256 + half * 128
                for kc in range(KCH):
                    nc.tensor.matmul(
                        ps,
                        lhsT=wqk_sb[:, kc, col:col + 128],
                        rhs=xt_sb[:, kc, g * 512:(g + 1) * 512],
                        start=(kc == 0), stop=(kc == KCH - 1),
                    )
                if biased:
                    nc.vector.tensor_scalar_add(
                        dest[:, half, g * 512:(g + 1) * 512],
                        ps, bq_sb[:, half:half + 1])
                else:
                    nc.vector.tensor_copy(
                        dest[:, half, g * 512:(g + 1) * 512], ps)

            def emit_v(stl):
                st = 4 * g + stl
                ps = ps_cp.tile([128, 512], F32, tag="cp", name=f"v{st}")
                for kc in range(KCH):
                    nc.tensor.matmul(
                        ps[:, 0:256],
                        lhsT=xt_sb[:, kc, st * 128:(st + 1) * 128],
                        rhs=wv_sb[:, kc, :],
                        start=(kc == 0), stop=(kc == KCH - 1),
                    )
                nc.vector.tensor_copy(v_sb[:, st, :], ps[:, 0:256])

            return ([lambda jt=0, h=h: emit_qk_half(0, h, qT, True)
                     for h in range(2)],
                    [lambda jt=1, h=h: emit_qk_half(1, h, kT, False)
                     for h in range(2)],
                    [lambda s=s: emit_v(s) for s in range(4)])

        def attention_round(g, fillers, dst, dst_off, ag=None):
            # fillers: list of (fn, need_by_kt); each fn MUST be emitted
            # before the kt it is needed by; the rest pace evenly.
            fillers = [f if isinstance(f, tuple) else (f, None)
                       for f in fillers]
            nf = len(fillers)
            nfdone = 0
            n_kt = 4 * (g + 1)
            av = ps_av.tile([128, 2, 512], F32, tag="av", name=f"av{g}")
            sums = ps_sum.tile([128, 512], F32, tag="sum", name=f"sum{g}")
            # only partitions {0,32,64,96} are matmul-written; define the
            # rest so the downstream full-tile reciprocal reads clean data
            nc.vector.memset(sums, 1.0)
            pts = {}
            for kt in range(n_kt):
                want = (kt + 1) * nf // n_kt
                while nfdone < nf and (
                        nfdone < want
                        or (fillers[nfdone][1] is not None
                            and fillers[nfdone][1] <= kt)):
                    fillers[nfdone][0]()
                    nfdone += 1
                qo = max((kt - 4 * g) * 128, 0)
                for pair in range(2):
                    sps = ps_m.tile([128, 2, 512], F32, tag="m",
                                    name=f"s{g}_{kt}_{pair}")
                    for hl in range(2):
                        nc.tensor.matmul(
                            sps[:, hl, qo:512],
                            lhsT=kT[hl * 64:(hl + 1) * 64, pair,
                                    kt * 128:(kt + 1) * 128],
                            rhs=qT[hl * 64:(hl + 1) * 64, pair,
                                   g * 512 + qo:(g + 1) * 512],
                            start=True, stop=True,
                            tile_position=(64 * hl, 0),
                        )
                    if kt >= 4 * g:  # diagonal: additive -30 mask pre-exp
                        nc.vector.tensor_add(
                            sps[:, :, qo:qo + 128], sps[:, :, qo:qo + 128],
                            mask_sb)
                    pt = pt_pool.tile([128, 2, 512], BF16, tag="pt",
                                      name=f"pt{g}_{kt}_{pair}")
                    nc.scalar.activation(
                        pt[:, :, qo:512], sps[:, :, qo:512], EXP)
                    pts[pair] = pt
                # AV: col-packed 2 heads per matmul slot
                for pair in range(2):
                    for hl in range(2):
                        nc.tensor.matmul(
                            av[64 * hl:64 * (hl + 1), pair, qo:512],
                            lhsT=v_sb[:, kt, (2 * pair + hl) * 64:
                                      (2 * pair + hl + 1) * 64],
                            rhs=pts[pair][:, hl, qo:512],
                            start=(kt == 0), stop=(kt == n_kt - 1),
                            tile_position=(0, 64 * hl),
                            skip_group_check=True,
                        )
                # denominators: 4 concurrent col-tiled M=1 ones-matmuls
                for h in range(4):
                    nc.tensor.matmul(
                        sums[32 * h:32 * h + 1, qo:512],
                        lhsT=ones_sb[:, 0:1],
                        rhs=pts[h // 2][:, h % 2, qo:512],
                        start=(kt == 0), stop=(kt == n_kt - 1),
                        tile_position=(0, 32 * h),
                        skip_group_check=True,
                    )
            while nfdone < nf:
                fillers[nfdone][0]()
                nfdone += 1
            # normalize producer-side. Copy raw ctx out of PSUM first so
            # the AV accumulator frees for the next round immediately; the
            # reciprocal is broadcast across partitions with K=1 ones
            # outer-product matmuls (all on-chip, no DRAM bounce).
            cs_raw = post.tile([128, 2, 512], BF16, tag="csr", name=f"csr{g}")
            nc.vector.tensor_copy(cs_raw, av)
            recip_p = post.tile([128, 512], F32, tag="rcp", name=f"rcp{g}")
            nc.vector.reciprocal_approx_fast(recip_p, sums)
            bc_ps = ps_m.tile([128, 2, 512], F32, tag="m", name=f"bc{g}")
            for pair in range(2):
                for hl in range(2):
                    h = 2 * pair + hl
                    nc.tensor.matmul(
                        bc_ps[64 * hl:64 * (hl + 1), pair, :],
                        lhsT=ones_row[32 * h:32 * h + 1, :],
                        rhs=recip_p[32 * h:32 * h + 1, :],
                        start=True, stop=True,
                        tile_position=(32 * h, 64 * hl),
                        skip_group_check=True,
                    )
            cs = post.tile([128, 2, 512], BF16, tag="cs", name=f"cs{g}")
            nc.vector.tensor_mul(cs, cs_raw, bc_ps)
            nc.sync.dma_start(
                out=bass.AP(tensor=dst.tensor,
                            offset=dst.offset + dst_off * 512,
                            ap=[[512, 128], [128 * 512, 2], [1, 512]]),
                in_=cs)
            if taps is not None:
                nc.sync.dma_start(out=taps["dbg_cs"][g], in_=cs)
                sums_f = post.tile([128, 512], F32, tag="dbgs", name=f"dbgs{g}")
                nc.vector.tensor_copy(sums_f, sums)
                nc.sync.dma_start(out=taps["dbg_sums"][g], in_=sums_f)
            if ag is not None:
                ag_in, ag_out = ag
                nc.gpsimd.collective_compute(
                    "AllGather", mybir.AluOpType.bypass, replica_groups=rg,
                    ins=[ag_in.opt()], outs=[ag_out.opt()],
                )

        def cproj_chunks(g, src, blk, base):
            state = {}
            dma = nc.scalar.dma_start

            def prologue():
                ctx_n = ctxg_pool.tile([128, KCH, 512], BF16, tag="cg",
                                       name=f"cg{g}")
                for r in range(4):
                    dma(out=ctx_n[:, 2 * r:2 * r + 2, :],
                        in_=bass.AP(tensor=src.tensor,
                                    offset=src.offset + (blk * r + base) * 512,
                                    ap=[[512, 128], [128 * 512, 2], [1, 512]]))
                state["ctx_n"] = ctx_n
                state["o"] = osb.tile([128, 2, 512], F32, tag="o", name=f"o{g}")

            def emit_cg(cg):
                ps = ps_cp.tile([128, 512], F32, tag="cp", name=f"cp{g}_{cg}")
                for kc in range(KCH):
                    nc.tensor.matmul(
                        ps,
                        lhsT=wp_sb[:, kc, cg * 128:(cg + 1) * 128],
                        rhs=state["ctx_n"][:, kc, :],
                        start=(kc == 0), stop=(kc == KCH - 1),
                    )
                nc.vector.tensor_scalar_add(state["o"][:, cg, :], ps,
                                            bp_sb[:, cg:cg + 1])
                if cg == 1:
                    dma(out=outT.rearrange("(a p) q -> p a q", p=128)[
                            :, :, g * 512:(g + 1) * 512],
                        in_=state["o"])

            return [prologue, lambda: emit_cg(0), lambda: emit_cg(1)]

        # ---- schedule ----
        q0, k0, v0 = qkv_chunks(0)
        q1, k1, v1 = qkv_chunks(1)
        q2, k2, v2 = qkv_chunks(2)
        q3, k3, v3 = qkv_chunks(3)
        for ch in qkv_chunks(0, pool=ps_m)[1] + qkv_chunks(1, pool=ps_m)[0]:
            ch()
        attention_round(
            1,
            [(v0[0], 0), (v0[1], 1), (v0[2], 2), (k1[0], 3), (v0[3], 3),
             (k1[1], 3), (v1[0], 4), (v1[1], 5), (v1[2], 6), (v1[3], 7),
             q2[0], q2[1]],
            ctx_local[1], 0, ag=(ctx_local[1], ctx_all[1]))
        attention_round(
            2,
            [(k2[0], 6), (k2[1], 7), (v2[0], 8), (v2[1], 9), (v2[2], 10),
             (v2[3], 11), k3[0], k3[1], q3[0], q3[1]],
            ctx_local[2], 0, ag=(ctx_local[2], ctx_all[2]))
        attention_round(
            3,
            [q0[0], q0[1], (v3[0], 12), (v3[1], 13), (v3[2], 14),
             (v3[3], 15)],
            ctx_local[3], 0, ag=(ctx_local[3], ctx_all[3]))
        attention_round(0, [], ctx_local[0], 0,
                        ag=(ctx_local[0], ctx_all[0]))
        # all cproj at the tail: gathers for supers 1/2 are long complete,
        # 3 completes under the first two, 0 under the next; loads ride the
        # idle Scalar engine so nothing blocks mid-attention streams.
        for g in (1, 2, 3, 0):
            for ch in cproj_chunks(g, ctx_all[g], 256, 0):
                ch()


_CACHE = {}


def _get_compiled():
    if "nc" not in _CACHE:
        nc = bacc.Bacc("TRN2", target_bir_lowering=False, debug=False,
                       num_devices=NCORES)
        build_ir(nc)
        nc.compile()
        _CACHE["nc"] = nc
    return _CACHE["nc"]


def make_in_maps(inputs):
    x = np.asarray(inputs["hidden_states"], dtype=np.float32)   # [B,S,D]
    wa = np.asarray(inputs["c_attn_w"], dtype=np.float32)       # [D, 3D]
    ba = np.asarray(inputs["c_attn_b"], dtype=np.float32)       # [3D]
    wpr = np.asarray(inputs["c_proj_w"], dtype=np.float32)      # [D, D]
    bpr = np.asarray(inputs["c_proj_b"], dtype=np.float32)      # [D]

    scale = 1.0 / (HD ** 0.5)
    wq = wa[:, 0:D] * scale
    wk = wa[:, D:2 * D]
    wv_full = wa[:, 2 * D:3 * D]
    bq_full = ba[0:D] * scale
    bv_full = ba[2 * D:3 * D]

    bf = ml_dtypes.bfloat16
    xTb = [np.ascontiguousarray(x[b].T.astype(bf)) for b in range(B)]

    in_maps = []
    for r in range(NCORES):
        b = r // 4
        hs = slice(256 * (r % 4), 256 * (r % 4) + 256)
        wp_slice = wpr[:, hs]
        in_maps.append({
            "xT": xTb[b],
            "wqk": np.ascontiguousarray(
                np.concatenate([wq[:, hs], wk[:, hs]], axis=1).astype(bf)),
            "wv": np.ascontiguousarray(wv_full[:, hs].astype(bf)),
            "wp": np.ascontiguousarray(wp_slice.astype(bf)),
            "bq": np.ascontiguousarray(bq_full[hs]),
            "bp": np.ascontiguousarray(bpr[hs] + bv_full @ wp_slice),
        })
    return in_maps


def assemble(results):
    out = np.empty((B, S, D), dtype=np.float32)
    for r in range(NCORES):
        b = r // 4
        hs = slice(256 * (r % 4), 256 * (r % 4) + 256)
        out[b, :, hs] = results[r]["outT"].T
    return out


def kernel(**inputs):
    in_maps = make_in_maps(inputs)
    nc = _get_compiled()
    res = run_bass_kernel_spmd(nc, in_maps, core_ids=list(range(NCORES)))
    return assemble(res.results)


if __name__ == "__main__":
    import reference
    inp = reference.setup_inputs()
    out = kernel(**{k: np.asarray(v) for k, v in inp.items()})
    print(out.shape, out.dtype)
